# revision 15
# baseline (speedup 1.0000x reference)
"""DeepseekV3 MoE MLP (grouped ragged GEMM) on 8 Trainium2 NeuronCores.

Strategy: expert-parallel. 32 experts / 8 cores = 4 experts per core; each
core processes its experts' token groups (tokens arrive pre-sorted by
expert). Compute in bf16 (fp32 accumulation in PSUM); output staged bf16 on
device, upcast to fp32 host-side.

Per-core pipeline, per expert (H=2048, I=1408, C tokens padded):
  stage 1:  gateT[i,t] = sum_h W1[h,i] * XT[h,t]   (W1 tile = lhsT, XT = rhs)
            upT  [i,t] = sum_h W2[h,i] * XT[h,t]
            h2T  [i,t] = silu(gateT) * upT          (ScalarE Silu + VectorE mul)
  stage 2:  down [t,h] = sum_i h2T[i,t] * W3[i,h]   (h2T tile = lhsT, W3 = rhs)

All operands are laid out host-side so every DMA is 128 partitions x
contiguous per-partition blocks; no on-device transposes anywhere.

XT is loaded in 4 chunks per expert so the first matmul chain starts after
~1MB of DMA instead of 2.5MB (trims the PE head gap). Output stores go
through the Activation-engine HWDGE ring to keep the SP ring free for loads.
"""

import numpy as np
import ml_dtypes

# Problem constants (hardcoded per contract).
E = 32          # experts
H = 2048        # hidden dim
I = 1408        # moe intermediate dim
N_CORES = 8
EPC = E // N_CORES  # experts per core
P = 128
HO = H // P     # 16 h-subtiles
IT = I // P     # 11 i-subtiles
HC = H // 512   # 4 output h-chunks of 512
XC = 4          # xt load chunks per expert
HOC = HO // XC  # h-subtiles per xt chunk

BF16 = ml_dtypes.bfloat16

_PROGRAM_CACHE: dict = {}


def _build_program(C: int):
    """Build + compile the per-core Bass program for per-expert capacity C
    (multiple of 512). Returns nc."""
    import concourse.bacc as bacc
    import concourse.mybir as mybir
    import concourse.tile as tile

    NT = C // 512   # stage-1 token chunks of 512
    TT = C // P     # stage-2 token tiles of 128

    nc = bacc.Bacc("TRN2", debug=False, num_devices=N_CORES)

    xt = nc.dram_tensor("xt", [EPC * NT, XC, P, HOC, 512], mybir.dt.bfloat16,
                        kind="ExternalInput").ap()
    w1 = nc.dram_tensor("w1", [EPC, IT, P, HO, P], mybir.dt.bfloat16,
                        kind="ExternalInput").ap()
    w2 = nc.dram_tensor("w2", [EPC, IT, P, HO, P], mybir.dt.bfloat16,
                        kind="ExternalInput").ap()
    w3 = nc.dram_tensor("w3", [EPC, HC, P, IT, 512], mybir.dt.bfloat16,
                        kind="ExternalInput").ap()
    out = nc.dram_tensor("out", [EPC * C, H], mybir.dt.bfloat16,
                         kind="ExternalOutput").ap()

    with tile.TileContext(nc) as tc:
        with (
            tc.tile_pool(name="xt_pool", bufs=2 * XC) as xt_pool,
            tc.tile_pool(name="w12_pool", bufs=4) as w12_pool,
            tc.tile_pool(name="whead_pool", bufs=XC) as whead_pool,
            tc.tile_pool(name="w3_pool", bufs=3) as w3_pool,
            tc.tile_pool(name="h2t_pool", bufs=2) as h2t_pool,
            tc.tile_pool(name="act_pool", bufs=3) as act_pool,
            tc.tile_pool(name="out_pool", bufs=4) as out_pool,
            tc.tile_pool(name="ps_g", bufs=2, space="PSUM") as ps_g,
            tc.tile_pool(name="ps_u", bufs=2, space="PSUM") as ps_u,
            tc.tile_pool(name="ps_d", bufs=3, space="PSUM") as ps_d,
        ):
            for e in range(EPC):
                # ---- load this expert's XT token chunks ----
                # xt_ch[tch][xc] is a [P, HOC, 512] tile covering h-subtiles
                # xc*HOC .. (xc+1)*HOC.
                xt_ch = [[None] * XC for _ in range(NT)]
                w1_head = w2_head = None
                if e == 0:
                    # First expert: split the it=0 weight tiles into XC
                    # sub-chunks and interleave the DMA issue order with the
                    # xt chunks in consumption order, so the first matmul
                    # starts after ~640KB of DMA instead of ~2.5MB.
                    for tch in range(NT):
                        for xc in range(XC):
                            xt_ch[tch][xc] = xt_pool.tile(
                                [P, HOC, 512], mybir.dt.bfloat16, tag="xt",
                                name=f"xt_{tch}_{xc}")
                    w1_head = [whead_pool.tile([P, HOC, P], mybir.dt.bfloat16,
                                               tag="w1h", name=f"w1h_{k}")
                               for k in range(XC)]
                    w2_head = [whead_pool.tile([P, HOC, P], mybir.dt.bfloat16,
                                               tag="w2h", name=f"w2h_{k}")
                               for k in range(XC)]

                    # it=0 consumption order (interleaved gate/up chains):
                    # gate MMs of chunk k need w1[k]+xt[k] (SP ring); the up
                    # MMs need w2[k] ~0.9us later (ACT ring, so the w2 loads
                    # don't delay the critical xt stream).
                    for k in range(XC):
                        nc.sync.dma_start(
                            out=w1_head[k][:],
                            in_=w1[e, 0][:, k * HOC:(k + 1) * HOC])
                        nc.sync.dma_start(out=xt_ch[0][k][:],
                                          in_=xt[e * NT, k])
                        nc.scalar.dma_start(
                            out=w2_head[k][:],
                            in_=w2[e, 0][:, k * HOC:(k + 1) * HOC])
                    for tch in range(1, NT):
                        for xc in range(XC):
                            nc.sync.dma_start(out=xt_ch[tch][xc][:],
                                              in_=xt[e * NT + tch, xc])
                else:
                    for tch in range(NT):
                        for xc in range(XC):
                            t_sb = xt_pool.tile([P, HOC, 512],
                                                mybir.dt.bfloat16, tag="xt")
                            nc.sync.dma_start(out=t_sb[:],
                                              in_=xt[e * NT + tch, xc])
                            xt_ch[tch][xc] = t_sb

                h2t = h2t_pool.tile([P, IT, C], mybir.dt.bfloat16, tag="h2t")

                # ---- stage 1: gateT/upT + silu*mul -> h2T ----
                for it in range(IT):
                    if e == 0 and it == 0:
                        w1_parts, w2_parts = w1_head, w2_head
                    else:
                        w1_sb = w12_pool.tile([P, HO, P], mybir.dt.bfloat16,
                                              tag="w1")
                        nc.sync.dma_start(out=w1_sb[:], in_=w1[e, it])
                        w2_sb = w12_pool.tile([P, HO, P], mybir.dt.bfloat16,
                                              tag="w2")
                        nc.sync.dma_start(out=w2_sb[:], in_=w2[e, it])
                        w1_parts = w2_parts = None

                    for tch in range(NT):
                        pg = ps_g.tile([P, 512], mybir.dt.float32, tag="pg")
                        pu = ps_u.tile([P, 512], mybir.dt.float32, tag="pu")
                        if w1_parts is not None:
                            # head iteration: interleave gate/up chains
                            # chunk-by-chunk to track DMA arrival order.
                            for k in range(XC):
                                for h2 in range(HOC):
                                    ho = k * HOC + h2
                                    nc.tensor.matmul(
                                        pg, w1_parts[k][:, h2],
                                        xt_ch[tch][k][:, h2],
                                        start=(ho == 0), stop=(ho == HO - 1))
                                for h2 in range(HOC):
                                    ho = k * HOC + h2
                                    nc.tensor.matmul(
                                        pu, w2_parts[k][:, h2],
                                        xt_ch[tch][k][:, h2],
                                        start=(ho == 0), stop=(ho == HO - 1))
                        else:
                            for ho in range(HO):
                                nc.tensor.matmul(pg, w1_sb[:, ho],
                                                 xt_ch[tch][ho // HOC][:, ho % HOC],
                                                 start=(ho == 0), stop=(ho == HO - 1))
                            for ho in range(HO):
                                nc.tensor.matmul(pu, w2_sb[:, ho],
                                                 xt_ch[tch][ho // HOC][:, ho % HOC],
                                                 start=(ho == 0), stop=(ho == HO - 1))
                        sil = act_pool.tile([P, 512], mybir.dt.float32, tag="sil")
                        nc.scalar.activation(sil, pg,
                                             mybir.ActivationFunctionType.Silu)
                        nc.vector.tensor_mul(
                            h2t[:, it, tch * 512:(tch + 1) * 512], sil, pu)

                # ---- stage 2: down = h2 @ W3 ----
                for hc in range(HC):
                    w3_sb = w3_pool.tile([P, IT, 512], mybir.dt.bfloat16, tag="w3")
                    nc.sync.dma_start(out=w3_sb[:], in_=w3[e, hc])
                    for tt in range(TT):
                        pd = ps_d.tile([P, 512], mybir.dt.float32, tag="pd")
                        for io in range(IT):
                            nc.tensor.matmul(
                                pd, h2t[:, io, tt * P:(tt + 1) * P], w3_sb[:, io],
                                start=(io == 0), stop=(io == IT - 1))
                        ot = out_pool.tile([P, 512], mybir.dt.bfloat16, tag="ot")
                        nc.scalar.copy(ot, pd)
                        nc.scalar.dma_start(
                            out=out[e * C + tt * P: e * C + (tt + 1) * P,
                                    hc * 512:(hc + 1) * 512],
                            in_=ot[:])

    nc.compile()
    return nc


def _prep_inputs(hidden_states, gate_w, up_w, down_w, group_sizes, C):
    """Host-side: group tokens by expert (padded to C), transpose, convert to
    bf16, and pre-tile everything into the DMA layouts the program expects.
    Returns (in_maps, offsets, gs)."""
    T = hidden_states.shape[0]
    gs = np.asarray(group_sizes, dtype=np.int64)
    offsets = np.zeros(E + 1, dtype=np.int64)
    np.cumsum(gs, out=offsets[1:])
    assert offsets[-1] == T, f"group_sizes sum {offsets[-1]} != T {T}"

    # Pad each expert's token block to C rows, convert to bf16.
    x_pad = np.zeros((E, C, H), dtype=BF16)
    for e in range(E):
        x_pad[e, :gs[e]] = hidden_states[offsets[e]:offsets[e + 1]]

    NT = C // 512
    # XT layout: [core][e_local*NT + tch][xc][p][hoc][512]
    # with h = (xc*HOC + hoc)*128 + p
    xt_all = np.ascontiguousarray(
        x_pad.reshape(E, NT, 512, XC, HOC, P).transpose(0, 1, 3, 5, 4, 2)
    ).reshape(N_CORES, EPC * NT, XC, P, HOC, 512)

    # W1/W2 layout: [E][it][p][ho][128i] with h = ho*128 + p
    def tile_w12(w):
        wb = np.asarray(w, dtype=BF16)
        return np.ascontiguousarray(
            wb.reshape(E, HO, P, IT, P).transpose(0, 3, 2, 1, 4)
        ).reshape(N_CORES, EPC, IT, P, HO, P)

    w1_all = tile_w12(gate_w)
    w2_all = tile_w12(up_w)

    # W3 layout: [E][hc][p][io][512h] with i = io*128 + p
    w3b = np.asarray(down_w, dtype=BF16)
    w3_all = np.ascontiguousarray(
        w3b.reshape(E, IT, P, HC, 512).transpose(0, 3, 2, 1, 4)
    ).reshape(N_CORES, EPC, HC, P, IT, 512)

    in_maps = [
        {"xt": xt_all[c], "w1": w1_all[c], "w2": w2_all[c], "w3": w3_all[c]}
        for c in range(N_CORES)
    ]
    return in_maps, offsets, gs


def _run(hidden_states, gate_w, up_w, down_w, group_sizes, trace=False):
    from concourse.bass_utils import run_bass_kernel_spmd

    gs = np.asarray(group_sizes, dtype=np.int64)
    max_g = int(gs.max()) if gs.size else 512
    C = max(512, -(-max_g // 512) * 512)  # round up to multiple of 512

    key = ("v2", C)
    if key not in _PROGRAM_CACHE:
        _PROGRAM_CACHE[key] = _build_program(C)
    nc = _PROGRAM_CACHE[key]

    in_maps, offsets, gs = _prep_inputs(
        hidden_states, gate_w, up_w, down_w, group_sizes, C)

    res = run_bass_kernel_spmd(nc, in_maps, core_ids=list(range(N_CORES)),
                               trace=trace)

    T = hidden_states.shape[0]
    out_full = np.empty((T, H), dtype=np.float32)
    for c in range(N_CORES):
        core_out = res.results[c]["out"]  # [EPC*C, H] bf16
        for el in range(EPC):
            e = c * EPC + el
            out_full[offsets[e]:offsets[e + 1]] = \
                core_out[el * C: el * C + gs[e]].astype(np.float32)
    return out_full, res.exec_time_ns


def kernel(hidden_states, gate_w, up_w, down_w, group_sizes):
    out, _ = _run(hidden_states, gate_w, up_w, down_w, group_sizes)
    return out
